# revision 1
# baseline (speedup 1.0000x reference)
"""Cross-attention Trainium2 Bass kernel.

Sharding: data-parallel over batch — 16 batches across 8 cores, 2 per core.
Weights replicated. Each core computes its 2 batches fully; no collectives.

Per-core dataflow (big matmuls in fp32r — 1 cycle/row at moving dim >= 256):
  - ctx^T via PE transpose; kT = Wk^T @ ctx^T; v = ctx @ Wv   (fp32: N=77 odd)
  - per 512-row tile of x:
      x^T via PE transposes -> q^T = Wq^T @ x^T                (fp32r)
      per head: scores^T = kT_h^T @ qT_h   [77, 512]           (fp32r)
                expT = exp(0.125 * scores^T)                   (ACT)
                attnU^T = v_h^T @ expT     [64, 512]           (fp32r)
                R = ones(77,64)^T @ expT   [64, 512] = denom   (fp32r)
                attnT_h = attnU * (1/R)                        (DVE)
      out = attnT^T @ Wout + bout                              (fp32r)

fp32r operand tiles must be written by a rounding instruction (ACT/DVE
convert-copy); fp32r matmul outputs must start at PSUM partition 0 and
have even moving dims. TRN2 allows 1 semaphore wait per instruction —
generate_event_semaphores() legalizes the multi-wait instructions Tile
emits.
"""

import numpy as np

import bass_rust as _bass_rust
import concourse.bass as bass
import concourse.mybir as mybir
import concourse.tile as tile
from concourse.bass_utils import run_bass_kernel_spmd
from concourse.masks import make_identity

N_CORES = 8
B, SQ, DM = 16, 4096, 512
SKV, DC = 77, 768
H, DH = 8, 64
INNER = 512
BPC = B // N_CORES  # batches per core

F32 = mybir.dt.float32
F32R = mybir.dt.float32r

AF = mybir.ActivationFunctionType


def build_nc(trace_sim=False):
    nc = bass.Bass()

    x_d = nc.dram_tensor("x", [BPC, SQ, DM], F32, kind="ExternalInput")
    ctx_d = nc.dram_tensor("context", [BPC, SKV, DC], F32, kind="ExternalInput")
    wq_d = nc.dram_tensor("Wq", [DM, INNER], F32, kind="ExternalInput")
    wk_d = nc.dram_tensor("Wk", [DC, INNER], F32, kind="ExternalInput")
    wv_d = nc.dram_tensor("Wv", [DC, INNER], F32, kind="ExternalInput")
    wo_d = nc.dram_tensor("Wout", [INNER, INNER], F32, kind="ExternalInput")
    bo_d = nc.dram_tensor("bout", [INNER], F32, kind="ExternalInput")
    out_d = nc.dram_tensor("out", [BPC, SQ, DM], F32, kind="ExternalOutput")

    with tile.TileContext(nc, trace_sim=trace_sim) as tc:
        with (
            tc.tile_pool(name="const", bufs=1) as consts,
            tc.tile_pool(name="wstage", bufs=2) as wstage,
            tc.tile_pool(name="perbatch", bufs=2) as pb,
            tc.tile_pool(name="work", bufs=2) as work,
            tc.tile_pool(name="exps", bufs=4) as exps,
            tc.tile_pool(name="smalls", bufs=6) as smalls,
            tc.tile_pool(name="pbig", bufs=2, space="PSUM") as pbig,
            tc.tile_pool(name="psc", bufs=2, space="PSUM") as psc,
            tc.tile_pool(name="pattU", bufs=2, space="PSUM") as pattU,
            tc.tile_pool(name="pR", bufs=2, space="PSUM") as pR,
        ):
            # ---- constants ----
            identity = consts.tile([128, 128], F32, tag="ident")
            make_identity(nc, identity)

            ones_stage = wstage.tile([SKV, DH], F32, tag="ones_stage")
            nc.vector.memset(ones_stage, 1.0)
            ones_t = consts.tile([SKV, DH], F32R, tag="ones")
            nc.scalar.copy(out=ones_t, in_=ones_stage)

            bias_b = consts.tile([128, INNER], F32, tag="bias")
            nc.gpsimd.dma_start(out=bias_b, in_=bo_d[:].partition_broadcast(128))

            # fp32r weights (Wq, Wout): DMA to staging, convert-copy rounds
            def load_w_f32r(dram, nchunk, tag):
                st = wstage.tile([128, nchunk, INNER], F32, tag="wstage")
                nc.sync.dma_start(out=st, in_=dram[:].rearrange("(c p) e -> p c e", p=128))
                wt = consts.tile([128, nchunk, INNER], F32R, tag=tag)
                nc.scalar.copy(out=wt, in_=st)
                return wt

            wq_sb = load_w_f32r(wq_d, DM // 128, "wq")
            wo_sb = load_w_f32r(wo_d, INNER // 128, "wo")

            # fp32 weights (Wk, Wv — k/v projections run in plain fp32)
            wk_sb = consts.tile([128, DC // 128, INNER], F32, tag="wk")
            nc.sync.dma_start(out=wk_sb, in_=wk_d[:].rearrange("(c p) e -> p c e", p=128))
            wv_sb = load_w_f32r(wv_d, DC // 128, "wv")

            def emit_outproj(attnT, b, s0):
                for t in range(4):
                    po = pbig.tile([128, 512], F32, tag="big")
                    for i in range(4):
                        nc.tensor.matmul(
                            out=po,
                            lhsT=attnT[:, i, t * 128:(t + 1) * 128],
                            rhs=wo_sb[:, i, :],
                            start=(i == 0), stop=(i == 3),
                        )
                    osb = smalls.tile([128, 512], F32, tag="osb")
                    nc.vector.tensor_add(osb, po, bias_b)
                    nc.sync.dma_start(
                        out=out_d[b, s0 + t * 128:s0 + (t + 1) * 128, :],
                        in_=osb,
                    )

            def emit_kv(b):
                # ---- context load + transpose (fp32) ----
                ctx_sb = pb.tile([SKV, DC], F32, tag="ctx")
                nc.sync.dma_start(out=ctx_sb, in_=ctx_d[b])

                ctxT = pb.tile([128, DC // 128, SKV], F32, tag="ctxT")
                for j in range(DC // 128):
                    pt = pbig.tile([128, 512], F32, tag="big")
                    nc.tensor.matmul(
                        out=pt[:, 0:SKV],
                        lhsT=ctx_sb[:, j * 128:(j + 1) * 128],
                        rhs=identity[0:SKV, 0:SKV],
                        is_transpose=True, start=True, stop=True,
                    )
                    nc.scalar.copy(out=ctxT[:, j, :], in_=pt[:, 0:SKV])
                # rounded copy of ctxT so the v projection can run fp32r
                ctxT_r = pb.tile([128, DC // 128, SKV], F32R, tag="ctxT_r")
                nc.scalar.copy(out=ctxT_r, in_=ctxT)

                # ---- kT = Wk^T @ ctx^T : [128e, 4, 77] (fp32 MM: N=77 odd) ----
                kT_sb = pb.tile([128, INNER // 128, SKV], F32R, tag="kT")
                for i in range(INNER // 128):
                    pk = pbig.tile([128, 512], F32, tag="big")
                    for j in range(DC // 128):
                        nc.tensor.matmul(
                            out=pk[:, 0:SKV],
                            lhsT=wk_sb[:, j, i * 128:(i + 1) * 128],
                            rhs=ctxT[:, j, :],
                            start=(j == 0), stop=(j == DC // 128 - 1),
                        )
                    nc.scalar.copy(out=kT_sb[:, i, :], in_=pk[:, 0:SKV])

                # ---- v = ctx @ Wv : [77, 512] (fp32r) ----
                v_sb = pb.tile([SKV, INNER], F32R, tag="v")
                pv = pbig.tile([128, 512], F32, tag="big")
                for j in range(DC // 128):
                    nc.tensor.matmul(
                        out=pv[0:SKV, :],
                        lhsT=ctxT_r[:, j, :],
                        rhs=wv_sb[:, j, :],
                        start=(j == 0), stop=(j == DC // 128 - 1),
                    )
                nc.scalar.copy(out=v_sb, in_=pv[0:SKV, :])
                return kT_sb, v_sb

            prev = None
            kv = emit_kv(0)
            for b in range(BPC):
                kT_sb, v_sb = kv
                for st in range(SQ // 512):
                    # emit next batch's k/v mid-batch so its PE work fills
                    # heads-phase stalls instead of serializing at the boundary
                    if st == 5 and b + 1 < BPC:
                        kv = emit_kv(b + 1)
                        kT_sb, v_sb = (kT_sb, v_sb)
                    s0 = st * 512
                    # ---- load x tile, transpose to xT ----
                    x_sb = work.tile([128, 4, DM], F32, tag="x")
                    nc.sync.dma_start(
                        out=x_sb,
                        in_=x_d[b, s0:s0 + 512, :].rearrange("(t p) d -> p t d", p=128),
                    )
                    xT = work.tile([128, 4, 512], F32R, tag="xT")  # [dm, dm_chunk, s]
                    for j in range(4):
                        pt = pbig.tile([128, 512], F32, tag="big")
                        for t in range(4):
                            nc.tensor.matmul(
                                out=pt[:, t * 128:(t + 1) * 128],
                                lhsT=x_sb[:, t, j * 128:(j + 1) * 128],
                                rhs=identity,
                                is_transpose=True,
                                start=(t == 0), stop=(t == 3),
                            )
                        nc.scalar.copy(out=xT[:, j, :], in_=pt)

                    # ---- out projection of the PREVIOUS tile (lag-1): its
                    # inputs are long ready, and emitting it early keeps the
                    # SP DMA stream from head-of-line blocking the next x load
                    if prev is not None:
                        emit_outproj(*prev)
                        prev = None

                    # ---- qT = Wq^T @ xT, with the two heads of each
                    # e-chunk emitted right after the chunk's copy ----
                    qT = work.tile([128, 4, 512], F32R, tag="qT")
                    attnT = work.tile([128, 4, 512], F32R, tag="attnT")
                    for i in range(4):
                        pq = pbig.tile([128, 512], F32, tag="big")
                        for j in range(4):
                            nc.tensor.matmul(
                                out=pq,
                                lhsT=wq_sb[:, j, i * 128:(i + 1) * 128],
                                rhs=xT[:, j, :],
                                start=(j == 0), stop=(j == 3),
                            )
                        nc.scalar.copy(out=qT[:, i, :], in_=pq)

                        # ---- attention for heads 2i, 2i+1 (fp32r) ----
                        for sub in range(2):
                            h, p, r0 = 2 * i + sub, i, sub * 64
                            ps = psc.tile([SKV, 512], F32, tag="sc")
                            nc.tensor.matmul(
                                out=ps,
                                lhsT=kT_sb[r0:r0 + 64, p, :],
                                rhs=qT[r0:r0 + 64, p, :],
                                start=True, stop=True,
                            )
                            et = exps.tile([SKV, 512], F32R, tag="expT")
                            nc.scalar.activation(
                                out=et, in_=ps, func=AF.Exp, scale=0.125,
                            )
                            pa = pattU.tile([64, 512], F32, tag="attnU")
                            nc.tensor.matmul(
                                out=pa,
                                lhsT=v_sb[:, h * 64:(h + 1) * 64],
                                rhs=et,
                                start=True, stop=True,
                            )
                            pr_ = pR.tile([64, 512], F32, tag="R")
                            nc.tensor.matmul(
                                out=pr_,
                                lhsT=ones_t,
                                rhs=et,
                                start=True, stop=True,
                            )
                            rr = smalls.tile([64, 512], F32, tag="rrec")
                            nc.vector.reciprocal(out=rr, in_=pr_)
                            nc.vector.tensor_mul(attnT[r0:r0 + 64, p, :], pa, rr)

                    prev = (attnT, b, s0)

            if prev is not None:
                emit_outproj(*prev)

    # TRN2 hardware allows at most 1 semaphore wait per instruction; split
    # multi-wait instructions into standalone EventSemaphore waits.
    _bass_rust.generate_event_semaphores(nc)
    return nc


_NC_CACHE = None


def kernel(x, context, Wq, Wk, Wv, Wout, bout):
    global _NC_CACHE
    if _NC_CACHE is None:
        _NC_CACHE = build_nc()
    nc = _NC_CACHE

    f = lambda a: np.ascontiguousarray(np.asarray(a), dtype=np.float32)
    x, context = f(x), f(context)
    Wq, Wk, Wv, Wout, bout = f(Wq), f(Wk), f(Wv), f(Wout), f(bout)

    in_maps = [
        {
            "x": x[c * BPC:(c + 1) * BPC],
            "context": context[c * BPC:(c + 1) * BPC],
            "Wq": Wq, "Wk": Wk, "Wv": Wv, "Wout": Wout, "bout": bout,
        }
        for c in range(N_CORES)
    ]
    res = run_bass_kernel_spmd(nc, in_maps, core_ids=list(range(N_CORES)))
    return np.concatenate([r["out"] for r in res.results], axis=0)



# revision 10
# speedup vs baseline: 1.1656x; 1.1656x over previous
"""Cross-attention Trainium2 Bass kernel.

Sharding: data-parallel over batch — 16 batches across 8 cores, 2 per core.
Weights replicated. Each core computes its 2 batches fully; no collectives.

All matmuls run in bf16 (1 cycle/row on TRN2 PE, fp32 PSUM accumulation;
tolerance budget is ~2e-2 so bf16's ~4e-3 rounding is safe).

Per-core dataflow, per 512-row x tile:
  - x -> bf16 (GPSIMD convert) -> x^T via DMA xbar transpose (no PE time)
  - q^T = Wq^T @ x^T                                   (PE, 16x512 rows)
  - per head: scores^T = kT_h^T @ qT_h   [77, 512]     (PE, 512 rows)
              expT = exp(0.125 * scores^T)             (ACT)
              [attnU^T; den] = [v_h | 1]^T @ expT      (PE, 512 rows)
                -> PSUM rows 0:64 = unnormalized attn, rows 64:128 = the
                   softmax denominator broadcast 64x (the ones-columns ride
                   along in the same matmul — no separate denominator matmul)
  - normalization on DVE in head PAIRS (same sub-row, adjacent qT chunks
    share one [128, 2x512] PSUM tile): one reciprocal [64,1024] PSUM->SBUF
    + one multiply [64,1024] -> attnT slice. GPSIMD cannot touch PSUM and
    TensorTensor may read only one PSUM operand, so recip+mul it is.
  - out projection: bias preloaded into PSUM by ACT, matmuls accumulate
    with start=False, result DMA'd PSUM->DRAM directly (no copy-out op).

Per-tile engine budget (2.4GHz PE): PE 24576 rows ~10.2us | ACT ~9.4us
(4 qT copies + 8 exp + 2 bias preloads) | DVE ~9.5us (4 recip + 4 mul) |
GPSIMD ~3us (x convert) | DMA ~7.6us. PE-bound.

TRN2 allows 1 semaphore wait per instruction — generate_event_semaphores()
legalizes the multi-wait instructions Tile emits.
"""

import numpy as np

import bass_rust as _bass_rust
import concourse.bass as bass
import concourse.mybir as mybir
import concourse.tile as tile
from concourse.bass_utils import run_bass_kernel_spmd
from concourse.masks import make_identity

N_CORES = 8
B, SQ, DM = 16, 4096, 512
SKV, DC = 77, 768
H, DH = 8, 64
INNER = 512
BPC = B // N_CORES  # batches per core

F32 = mybir.dt.float32
BF16 = mybir.dt.bfloat16

AF = mybir.ActivationFunctionType


def build_nc(trace_sim=False):
    nc = bass.Bass()

    x_d = nc.dram_tensor("x", [BPC, SQ, DM], F32, kind="ExternalInput")
    ctx_d = nc.dram_tensor("context", [BPC, SKV, DC], F32, kind="ExternalInput")
    wq_d = nc.dram_tensor("Wq", [DM, INNER], F32, kind="ExternalInput")
    wk_d = nc.dram_tensor("Wk", [DC, INNER], F32, kind="ExternalInput")
    wv_d = nc.dram_tensor("Wv", [DC, INNER], F32, kind="ExternalInput")
    wo_d = nc.dram_tensor("Wout", [INNER, INNER], F32, kind="ExternalInput")
    bo_d = nc.dram_tensor("bout", [INNER], F32, kind="ExternalInput")
    out_d = nc.dram_tensor("out", [BPC, SQ, DM], F32, kind="ExternalOutput")

    with tile.TileContext(nc, trace_sim=trace_sim) as tc:
        with (
            tc.tile_pool(name="const", bufs=1) as consts,
            tc.tile_pool(name="wstage", bufs=2) as wstage,
            tc.tile_pool(name="perbatch", bufs=2) as pb,
            tc.tile_pool(name="work", bufs=3) as work,
            tc.tile_pool(name="exps", bufs=4) as exps,
            tc.tile_pool(name="smalls", bufs=4) as smalls,
            tc.tile_pool(name="osbp", bufs=2) as osbp,
            tc.tile_pool(name="pbig", bufs=2, space="PSUM") as pbig,
            tc.tile_pool(name="psc", bufs=2, space="PSUM") as psc_p,
            tc.tile_pool(name="pu", bufs=2, space="PSUM") as pu_p,
        ):
            # ---- constants ----
            identity = consts.tile([128, 128], F32, tag="ident")
            make_identity(nc, identity)

            # bias enters the out-proj PSUM accumulation as a rank-1 matmul:
            # ones_row^T @ bout_row (512 moving rows, ~0.2us) — PSUM can't be
            # DMA'd or GPSIMD'd, so this keeps the bias off ACT/DVE entirely.
            ones_row = consts.tile([1, 128], BF16, tag="ones_row")
            nc.vector.memset(ones_row, 1.0)
            bo_st = wstage.tile([1, INNER], F32, tag="bo_st")
            nc.sync.dma_start(out=bo_st, in_=bo_d[:].partition_broadcast(1))
            bout_row = consts.tile([1, INNER], BF16, tag="bout_row")
            nc.vector.tensor_copy(out=bout_row, in_=bo_st)

            # bf16 weights, layout [p, c, e] with row index = c*128 + p —
            # matches both the DMA-xbar x^T layout and attnT's chunk layout.
            def load_w_bf16(dram, nchunk, tag, conv):
                st = wstage.tile([128, nchunk, INNER], F32, tag="wstage")
                nc.sync.dma_start(
                    out=st, in_=dram[:].rearrange("(c p) e -> p c e", p=128)
                )
                wt = consts.tile([128, nchunk, INNER], BF16, tag=tag)
                if conv == "act":
                    nc.scalar.copy(out=wt, in_=st)
                else:
                    nc.vector.tensor_copy(out=wt, in_=st)
                return wt

            wq_sb = load_w_bf16(wq_d, DM // 128, "wq", "act")
            wk_sb = load_w_bf16(wk_d, DC // 128, "wk", "act")
            wv_sb = load_w_bf16(wv_d, DC // 128, "wv", "dve")
            wo_sb = load_w_bf16(wo_d, INNER // 128, "wo", "dve")

            def emit_outproj(attnT, b, s0):
                # two [128, 2x512] PSUM pairs; bias rides in as a rank-1
                # accumulating matmul; one paired ACT copy-out per pair
                for tp in range(2):
                    po = pu_p.tile([128, 2, 512], F32, tag="u")
                    for t01 in range(2):
                        t = 2 * tp + t01
                        for c in range(4):
                            nc.tensor.matmul(
                                out=po[:, t01, :],
                                lhsT=attnT[:, c, t * 128:(t + 1) * 128],
                                rhs=wo_sb[:, c, :],
                                start=(c == 0), stop=False,
                            )
                        nc.tensor.matmul(
                            out=po[:, t01, :],
                            lhsT=ones_row,
                            rhs=bout_row,
                            start=False, stop=True,
                        )
                    osb = osbp.tile([128, 2, 512], F32, tag="osb")
                    nc.scalar.copy(out=osb, in_=po)
                    nc.sync.dma_start(
                        out=out_d[b, s0 + tp * 256:s0 + (tp + 1) * 256, :]
                        .rearrange("(t p) d -> p t d", p=128),
                        in_=osb,
                    )

            def emit_kv(b):
                # ---- context load + fp32 PE transpose (cheap: 6x77 rows) ----
                ctx_sb = pb.tile([SKV, DC], F32, tag="ctx")
                nc.sync.dma_start(out=ctx_sb, in_=ctx_d[b])

                ctxT = pb.tile([128, DC // 128, SKV], BF16, tag="ctxT")
                for j in range(DC // 128):
                    pt = pbig.tile([128, 512], F32, tag="big")
                    nc.tensor.matmul(
                        out=pt[:, 0:SKV],
                        lhsT=ctx_sb[:, j * 128:(j + 1) * 128],
                        rhs=identity[0:SKV, 0:SKV],
                        is_transpose=True, start=True, stop=True,
                    )
                    nc.scalar.copy(out=ctxT[:, j, :], in_=pt[:, 0:SKV])

                # ---- kT = Wk^T @ ctx^T : [128e, 4, 77] ----
                kT = pb.tile([128, INNER // 128, SKV], BF16, tag="kT")
                for i in range(INNER // 128):
                    pk = pbig.tile([128, 512], F32, tag="big")
                    for j in range(DC // 128):
                        nc.tensor.matmul(
                            out=pk[:, 0:SKV],
                            lhsT=wk_sb[:, j, i * 128:(i + 1) * 128],
                            rhs=ctxT[:, j, :],
                            start=(j == 0), stop=(j == DC // 128 - 1),
                        )
                    nc.scalar.copy(out=kT[:, i, :], in_=pk[:, 0:SKV])

                # ---- v_aug[:, h, :] = [v_h | ones] : [77, 8, 128] ----
                v_aug = pb.tile([SKV, H, 128], BF16, tag="vaug")
                nc.vector.memset(v_aug[:, :, 64:128], 1.0)
                pv = pbig.tile([128, 512], F32, tag="big")
                for j in range(DC // 128):
                    nc.tensor.matmul(
                        out=pv[0:SKV, :],
                        lhsT=ctxT[:, j, :],
                        rhs=wv_sb[:, j, :],
                        start=(j == 0), stop=(j == DC // 128 - 1),
                    )
                nc.scalar.copy(
                    out=v_aug[:, :, 0:64],
                    in_=pv[0:SKV, :].rearrange("k (h d) -> k h d", h=H),
                )
                return kT, v_aug

            prev = None
            kv = emit_kv(0)
            for b in range(BPC):
                kT, v_aug = kv
                for st in range(SQ // 512):
                    # emit next batch's k/v mid-batch so its PE work fills
                    # stalls instead of serializing at the boundary
                    if st == 5 and b + 1 < BPC:
                        kv = emit_kv(b + 1)
                    s0 = st * 512
                    # ---- load x tile, bf16 convert, DMA-xbar transpose ----
                    x_sb = work.tile([128, 4, DM], F32, tag="x")
                    nc.sync.dma_start(
                        out=x_sb,
                        in_=x_d[b, s0:s0 + 512, :].rearrange(
                            "(t p) d -> p t d", p=128
                        ),
                    )
                    x_bf = work.tile([128, 4, DM], BF16, tag="xbf")
                    nc.gpsimd.tensor_copy(out=x_bf, in_=x_sb)
                    xT = work.tile([128, 4, 512], BF16, tag="xT")  # d = c*128+p
                    for t in range(4):
                        nc.sync.dma_start_transpose(
                            out=xT[:, :, t * 128:(t + 1) * 128],
                            in_=x_bf[:, t, :],
                        )

                    # ---- out projection of the PREVIOUS tile (lag-1): its
                    # inputs are long ready; keeps PE fed while this tile's
                    # x^T transposes are still in flight
                    if prev is not None:
                        emit_outproj(*prev)
                        prev = None

                    # ---- per chunk pair: qT chunks, then head quads ----
                    qT = work.tile([128, 4, 512], BF16, tag="qT")
                    attnT = work.tile([128, 4, 512], BF16, tag="attnT")
                    for cp in range(2):
                        for i in (2 * cp, 2 * cp + 1):
                            pq = pbig.tile([128, 512], F32, tag="big")
                            for c in range(4):
                                nc.tensor.matmul(
                                    out=pq,
                                    lhsT=wq_sb[:, c, i * 128:(i + 1) * 128],
                                    rhs=xT[:, c, :],
                                    start=(c == 0), stop=(c == 3),
                                )
                            nc.scalar.copy(out=qT[:, i, :], in_=pq)

                        # heads for chunks (2cp, 2cp+1): sub=0 -> rows 0:64,
                        # sub=1 -> rows 64:128 of those qT/kT/attnT chunks
                        for sub in range(2):
                            r0 = sub * 64
                            pa = pu_p.tile([128, 2, 512], F32, tag="u")
                            for half in range(2):
                                i = 2 * cp + half
                                h = 2 * i + sub
                                ps = psc_p.tile([SKV, 512], F32, tag="sc")
                                nc.tensor.matmul(
                                    out=ps,
                                    lhsT=kT[r0:r0 + 64, i, :],
                                    rhs=qT[r0:r0 + 64, i, :],
                                    start=True, stop=True,
                                )
                                et = exps.tile([SKV, 512], BF16, tag="expT")
                                nc.scalar.activation(
                                    out=et, in_=ps, func=AF.Exp, scale=0.125,
                                )
                                nc.tensor.matmul(
                                    out=pa[:, half, :],
                                    lhsT=v_aug[:, h, :],
                                    rhs=et,
                                    start=True, stop=True,
                                )
                            rr = smalls.tile([64, 2, 512], F32, tag="rr")
                            nc.vector.reciprocal(out=rr, in_=pa[64:128, :, :])
                            nc.vector.tensor_mul(
                                attnT[r0:r0 + 64, 2 * cp:2 * cp + 2, :],
                                pa[0:64, :, :],
                                rr,
                            )

                    prev = (attnT, b, s0)

            if prev is not None:
                emit_outproj(*prev)

    # TRN2 hardware allows at most 1 semaphore wait per instruction; split
    # multi-wait instructions into standalone EventSemaphore waits.
    _bass_rust.generate_event_semaphores(nc)
    return nc


_NC_CACHE = None


def kernel(x, context, Wq, Wk, Wv, Wout, bout):
    global _NC_CACHE
    if _NC_CACHE is None:
        _NC_CACHE = build_nc()
    nc = _NC_CACHE

    f = lambda a: np.ascontiguousarray(np.asarray(a), dtype=np.float32)
    x, context = f(x), f(context)
    Wq, Wk, Wv, Wout, bout = f(Wq), f(Wk), f(Wv), f(Wout), f(bout)

    in_maps = [
        {
            "x": x[c * BPC:(c + 1) * BPC],
            "context": context[c * BPC:(c + 1) * BPC],
            "Wq": Wq, "Wk": Wk, "Wv": Wv, "Wout": Wout, "bout": bout,
        }
        for c in range(N_CORES)
    ]
    res = run_bass_kernel_spmd(nc, in_maps, core_ids=list(range(N_CORES)))
    return np.concatenate([r["out"] for r in res.results], axis=0)
